# revision 24
# baseline (speedup 1.0000x reference)
"""Trainium2 Bass kernel for modality-routed (CogVLM-style) attention.

Contract: kernel(**inputs) takes FULL unsharded numpy inputs (as produced by
the reference's setup_inputs) and returns the FULL [2048, 4096] fp32 output.

Sharding: tensor-parallel over heads. Core r owns heads 4r..4r+3:
  - qkv weights column-sharded; q/k computed weight-stationary producing
    qT/kT [d, tok] directly, v computed token-stationary in [tok, d].
  - dense weights row-sharded [512, 4096]; each core emits a partial
    [2048, 4096] fp32 output, summed on the host (the unshard step).

All PE work in bf16 (fp32 PSUM accumulation; bf16 output partials summed
in fp64 on the host). All intermediates (qT/kT, v, attn) stay SBUF-resident.
Host pretiles every tensor into its exact SBUF layout so each weight/hst
load is one contiguous-per-partition descriptor on a hardware-DGE queue
(sync/scalar; gpsimd software-DGE is ~50 GB/s and only runs softmax-sum
accumulation). Softmax: no max-subtraction (scores are O(10)); row sums
via gpsimd-accumulated probs + one all-ones fp32r matmul per (h, c-chunk)
whose PE output broadcasts the sums to all partitions; reciprocal via the
fast-approx DVE op; diagonal causal blocks compute only the surviving
columns. The 64 vision side tokens (512..576) run through moving-operand
matmuls + a PE transpose instead of 288 tiny 64-wide matmuls. Dense-phase
matmuls are interleaved one-by-one into attention's exp-wait bubbles
(per-chunk attn tiles avoid false cross-phase dependencies).
"""

import math
import sys

import numpy as np

if "/opt/trn_rl_repo" not in sys.path:
    sys.path.insert(0, "/opt/trn_rl_repo")

import ml_dtypes  # noqa: E402

import concourse.bass as bass  # noqa: E402,F401
import concourse.tile as tile  # noqa: E402
from concourse import bacc, mybir  # noqa: E402
from concourse.bass_utils import run_bass_kernel_spmd  # noqa: E402

S = 2048
HID = 4096
H = 32
D = 128
NCORES = 8
HPC = H // NCORES          # heads per core = 4
VC = HPC * D               # per-core q (or k or v) width = 512
NV = 576                   # vision tokens occupy rows [0, NV)
NKT = HID // 128           # 32 contraction tiles
SCALE = 1.0 / math.sqrt(D)

BF = mybir.dt.bfloat16
F32 = mybir.dt.float32
F32R = mybir.dt.float32r
BF_NP = ml_dtypes.bfloat16
AF = mybir.ActivationFunctionType

_CACHE = {}

CHUNKS = [(0, 512, "V", True), (NV, 1024, "L", False),
          (1024, 1536, "L", False), (1536, 2048, "L", False)]


def _token_tiles(t0, t1):
    out = []
    c = t0
    while c < t1:
        n = min(t1, (c // 128 + 1) * 128)
        out.append((c, n))
        c = n
    return out


def _build():
    nc = bacc.Bacc("TRN2", target_bir_lowering=False, debug=False,
                   num_devices=NCORES)
    dti = nc.dram_tensor
    hsq = dti("hsq", [4, 4, 128, 8, 576], BF, kind="ExternalInput").ap()
    wqk_v = dti("wqk_v", [8, 128, NKT, 128], BF, kind="ExternalInput").ap()
    wqk_l = dti("wqk_l", [8, 128, NKT, 128], BF, kind="ExternalInput").ap()
    wv_v = dti("wv_v", [128, NKT, VC], BF, kind="ExternalInput").ap()
    wv_l = dti("wv_l", [128, NKT, VC], BF, kind="ExternalInput").ap()
    wqm = dti("wqm", [8, 128, 4, 1024], BF, kind="ExternalInput").ap()
    wdw = dti("wdw", [2, 128, 2, HPC, 2048], BF, kind="ExternalInput").ap()
    bqk = dti("bqk", [128, 8], F32, kind="ExternalInput").ap()
    bv = dti("bv", [1, VC], F32, kind="ExternalInput").ap()
    cosw = dti("cosw", [D, S], BF, kind="ExternalInput").ap()
    sinw = dti("sinw", [D, S], BF, kind="ExternalInput").ap()
    rmT = dti("rmT", [D, D], BF, kind="ExternalInput").ap()
    idm = dti("idm", [64, 64], BF, kind="ExternalInput").ap()
    ones = dti("ones", [128, 128], BF, kind="ExternalInput").ap()
    ones_r = dti("ones_r", [128, 128], F32R, kind="ExternalInput").ap()
    masks = dti("masks", [128, 4, 512], BF, kind="ExternalInput").ap()
    out_d = dti("out", [S, HID], BF, kind="ExternalOutput").ap()

    with tile.TileContext(nc) as tc:
        with tc.tile_pool(name="glob", bufs=1) as glob:
            qk = glob.tile([128, 8, S], BF)        # qT (m 0..3) / kT (m 4..7)
            vsb = glob.tile([128, 16, VC], BF)     # v[128t+p, :] token tiles

            # ---------------- QKV phase ----------------
            with tc.tile_pool(name="hsA", bufs=2) as hsA_pool, \
                 tc.tile_pool(name="hsB", bufs=2) as hsB_pool, \
                 tc.tile_pool(name="hsC", bufs=2) as hsC_pool, \
                 tc.tile_pool(name="hsD", bufs=2) as hsD_pool, \
                 tc.tile_pool(name="cns", bufs=1) as cns, \
                 tc.tile_pool(name="wq", bufs=2) as wq_pool, \
                 tc.tile_pool(name="wvp", bufs=1) as wv_pool, \
                 tc.tile_pool(name="wqmp", bufs=2) as wqm_pool, \
                 tc.tile_pool(name="ev", bufs=2) as ev_pool, \
                 tc.tile_pool(name="psA", bufs=2, space="PSUM") as psA, \
                 tc.tile_pool(name="psR", bufs=2, space="PSUM") as psR, \
                 tc.tile_pool(name="psSd", bufs=1, space="PSUM") as psSd, \
                 tc.tile_pool(name="psT", bufs=1, space="PSUM") as psT:
                # first chunk's activations + weights land first (parallel
                # queues); consts follow on the vector queue.
                hs_tiles = []

                def load_hst(ci):
                    (c0, c1, _e, side) = CHUNKS[ci]
                    ww = (c1 - c0) + (64 if side else 0)
                    ha = hsA_pool.tile([128, 8, 576], BF, tag="hsA")
                    hb = hsB_pool.tile([128, 8, 576], BF, tag="hsB")
                    hc = hsC_pool.tile([128, 8, 576], BF, tag="hsC")
                    hd = hsD_pool.tile([128, 8, 576], BF, tag="hsD")
                    for qi, hq in enumerate((ha, hb, hc, hd)):
                        nc.sync.dma_start(out=hq[:, :, :ww],
                                          in_=hsq[ci, qi, :, :, :ww])
                    hs_tiles.append((ha, hb, hc, hd))

                for ci, (c0, c1, e, side) in enumerate(CHUNKS[:1]):
                    wqk = wqk_v if e == "V" else wqk_l
                    if ci == 0:
                        wt0 = wq_pool.tile([128, NKT, 128], BF, tag="wt")
                        nc.sync.dma_start(out=wt0[:, 0:8, :],
                                          in_=wqk[0][:, 0:8, :])
                        load_hst(0)
                        nc.sync.dma_start(out=wt0[:, 8:32, :],
                                          in_=wqk[0][:, 8:32, :])
                        load_hst(1)
                        cos_t = cns.tile([D, S], BF)
                        nc.scalar.dma_start(out=cos_t[:], in_=cosw[:])
                        sin_t = cns.tile([D, S], BF)
                        nc.scalar.dma_start(out=sin_t[:], in_=sinw[:])
                        rm_t = cns.tile([D, D], BF)
                        nc.scalar.dma_start(out=rm_t[:], in_=rmT[:])
                        idm_t = cns.tile([64, 64], BF)
                        nc.scalar.dma_start(out=idm_t[:], in_=idm[:])
                        bqk_t = cns.tile([128, 8], F32)
                        nc.scalar.dma_start(out=bqk_t[:], in_=bqk[:])
                        bv_t = cns.tile([128, VC], F32)
                        nc.scalar.dma_start(out=bv_t[:],
                                            in_=bv[:].to_broadcast((128, VC)))

                def hsrc(ci, kt):
                    return hs_tiles[ci][kt // 8], kt % 8

                for ci, (c0, c1, e, side) in enumerate(CHUNKS):
                    w = c1 - c0
                    wqk = wqk_v if e == "V" else wqk_l
                    wv = wv_v if e == "V" else wv_l
                    if ci + 2 < len(CHUNKS):
                        load_hst(ci + 2)
                    # --- q/k, weight-stationary -> qT/kT [d, tok] + RoPE ---
                    wvt = None
                    for m in range(8):
                        if ci == 0 and m == 0:
                            wt = wt0
                        else:
                            wt = wq_pool.tile([128, NKT, 128], BF, tag="wt")
                            nc.sync.dma_start(out=wt[:], in_=wqk[m])
                        if m == 3:
                            # v weights: prefetch mid m-loop (prior chunk's v
                            # reads are long done -> no queue stall)
                            wvt = wv_pool.tile([128, NKT, VC], BF, tag="wvt")
                            nc.scalar.dma_start(out=wvt[:], in_=wv[:])
                        pt = psA.tile([128, 512], F32, tag="pt")
                        for kt in range(NKT):
                            hs_t, k2 = hsrc(ci, kt)
                            nc.tensor.matmul(pt[:, :w], wt[:, kt, :],
                                             hs_t[:, k2, :w],
                                             start=(kt == 0),
                                             stop=(kt == NKT - 1))
                        qs = ev_pool.tile([128, 512], BF, tag="qs")
                        if e == "V":
                            nc.scalar.activation(out=qs[:, :w], in_=pt[:, :w],
                                                 func=AF.Identity,
                                                 bias=bqk_t[:, m:m + 1],
                                                 scale=1.0)
                        else:
                            nc.scalar.activation(out=qs[:, :w], in_=pt[:, :w],
                                                 func=AF.Copy, scale=1.0)
                        rot = psR.tile([128, 512], F32, tag="rot")
                        nc.tensor.matmul(rot[:, :w], rm_t[:], qs[:, :w],
                                         start=True, stop=True)
                        tb = ev_pool.tile([128, 512], BF, tag="tb")
                        nc.vector.tensor_mul(tb[:, :w], rot[:, :w],
                                             sin_t[:, c0:c1])
                        qc = ev_pool.tile([128, 512], BF, tag="qc")
                        nc.vector.tensor_mul(qc[:, :w], qs[:, :w],
                                             cos_t[:, c0:c1])
                        nc.vector.tensor_add(qk[:, m, c0:c1], qc[:, :w],
                                             tb[:, :w])
                    # --- v, token-stationary -> v [tok, d] ---
                    for (t0, t1) in _token_tiles(c0, c1):
                        mw = t1 - t0
                        pv = psA.tile([128, 512], F32, tag="pt")
                        for kt in range(NKT):
                            hs_t, k2 = hsrc(ci, kt)
                            nc.tensor.matmul(
                                pv[:mw, :], hs_t[:, k2, t0 - c0:t1 - c0],
                                wvt[:, kt, :],
                                start=(kt == 0), stop=(kt == NKT - 1))
                        tt, po = t0 // 128, t0 % 128
                        if po == 0:
                            if e == "V":
                                nc.vector.tensor_add(vsb[:mw, tt, :],
                                                     pv[:mw, :], bv_t[:mw, :])
                            else:
                                nc.scalar.activation(out=vsb[:mw, tt, :],
                                                     in_=pv[:mw, :],
                                                     func=AF.Copy, scale=1.0)
                        else:
                            # tokens 576..640: partition-offset fixup via DMA
                            vs = ev_pool.tile([128, 512], BF, tag="vs")
                            nc.scalar.activation(out=vs[:mw, :],
                                                 in_=pv[:mw, :],
                                                 func=AF.Copy, scale=1.0)
                            nc.gpsimd.dma_start(out=vsb[po:po + mw, tt, :],
                                                in_=vs[:mw, :])
                    # --- vision side tokens 512..576: moving-operand qkv ---
                    if side:
                        psd = []
                        for p in range(3):
                            psd_p = psSd.tile([128, 512], F32, tag=f"psd{p}")
                            psd.append(psd_p)
                        for ktc in range(8):
                            wq_c = wqm_pool.tile([128, 4, 1024], BF,
                                                 tag="wq_c")
                            nc.sync.dma_start(out=wq_c[:], in_=wqm[ktc])
                            for k8 in range(4):
                                kt = 4 * ktc + k8
                                hs_t, k2 = hsrc(ci, kt)
                                st = hs_t[:, k2, 512:576]
                                nc.tensor.matmul(
                                    psd[0][:64, :], st, wq_c[:, k8, 0:512],
                                    start=(kt == 0), stop=(kt == NKT - 1))
                                nc.tensor.matmul(
                                    psd[1][:64, :], st, wq_c[:, k8, 512:1024],
                                    start=(kt == 0), stop=(kt == NKT - 1))
                                wv_mv = wvt[:, kt, :]
                                nc.tensor.matmul(
                                    psd[2][:64, :], st, wv_mv,
                                    start=(kt == 0), stop=(kt == NKT - 1))
                        # v side: bias along free dim, straight into vsb
                        nc.vector.tensor_add(vsb[:64, 4, :], psd[2][:64, :],
                                             bv_t[:64, :])
                        # q/k side: evac, transpose to [d, tok], rope
                        sqk = ev_pool.tile([128, 8, 128], BF, tag="sqk")
                        nc.scalar.activation(out=sqk[:64, 0:4, :],
                                             in_=psd[0][:64, :],
                                             func=AF.Copy, scale=1.0)
                        nc.scalar.activation(out=sqk[:64, 4:8, :],
                                             in_=psd[1][:64, :],
                                             func=AF.Copy, scale=1.0)
                        for m in range(8):
                            tp = psT.tile([128, 64], BF, tag="tp")
                            nc.tensor.transpose(tp[:], sqk[:64, m, :],
                                                idm_t[:])
                            qs2 = ev_pool.tile([128, 64], BF, tag="qs2")
                            nc.scalar.activation(out=qs2[:], in_=tp[:],
                                                 func=AF.Identity,
                                                 bias=bqk_t[:, m:m + 1],
                                                 scale=1.0)
                            rot2 = psR.tile([128, 512], F32, tag="rot")
                            nc.tensor.matmul(rot2[:, :64], rm_t[:], qs2[:],
                                             start=True, stop=True)
                            tb2 = ev_pool.tile([128, 64], BF, tag="tb2")
                            nc.vector.tensor_mul(tb2[:], rot2[:, :64],
                                                 sin_t[:, 512:NV])
                            qc2 = ev_pool.tile([128, 64], BF, tag="qc2")
                            nc.vector.tensor_mul(qc2[:], qs2[:],
                                                 cos_t[:, 512:NV])
                            nc.vector.tensor_add(qk[:, m, 512:NV], qc2[:],
                                                 tb2[:])

            # ------------- attention + dense phases -------------
            with tc.tile_pool(name="wd", bufs=1) as wd_pool, \
                 tc.tile_pool(name="an", bufs=1) as an_pool:
                # prefetch dense weights during attention
                wd_t = wd_pool.tile([128, 2, 2, HPC, 2048], BF, tag="wd")
                nc.scalar.dma_start(out=wd_t[:, 0], in_=wdw[0])
                nc.scalar.dma_start(out=wd_t[:, 1], in_=wdw[1])
                attn_c = []
                for cc in range(4):
                    attn_t = an_pool.tile([128, HPC, 512], BF, tag=f"at{cc}")
                    attn_c.append(attn_t)
                with tc.tile_pool(name="acns", bufs=1) as acns, \
                     tc.tile_pool(name="pr", bufs=3) as pr_pool, \
                     tc.tile_pool(name="pa", bufs=2) as pa_pool, \
                     tc.tile_pool(name="sm", bufs=2) as sm_pool, \
                     tc.tile_pool(name="oe", bufs=3) as oe_pool, \
                     tc.tile_pool(name="psS", bufs=3, space="PSUM") as psS, \
                     tc.tile_pool(name="psP", bufs=2, space="PSUM") as psP, \
                     tc.tile_pool(name="psU", bufs=1, space="PSUM") as psU, \
                     tc.tile_pool(name="psD", bufs=1, space="PSUM") as psD:
                    mask_t = acns.tile([128, 4, 512], BF)
                    nc.scalar.dma_start(out=mask_t[:], in_=masks[:])
                    ones_t = acns.tile([128, 128], F32R)
                    nc.scalar.dma_start(out=ones_t[:], in_=ones_r[:])
                    for _ in range(3):
                        pb0 = pr_pool.tile([128, 512], BF, tag="pb")
                        nc.vector.memset(pb0[:], 0.0)

                    # dense work: one unit = one hh-matmul of a po group
                    # ((range, nh, n) accumulated over hh); interleaved into
                    # attention's exp-wait bubbles once the needed attn
                    # token-chunk is complete.
                    ranges = []
                    for (t0, t1) in _token_tiles(0, S):
                        if t0 < NV < t1:
                            ranges.append((t0, NV, 0))
                            ranges.append((NV, t1, 1))
                        else:
                            ranges.append((t0, t1, 0 if t0 < NV else 1))
                    units = []
                    for (t0, t1, ei) in ranges:
                        for nh in range(2):
                            for n in range(4):
                                for hh in range(HPC):
                                    units.append((t0 // 512, t0, t1, ei,
                                                  nh, n, hh))
                    dstate = {"gi": 0, "po": None, "ob": None}

                    def dense_step():
                        i = dstate["gi"]
                        if i >= len(units):
                            return False
                        (_, t0, t1, ei, nh, n, hh) = units[i]
                        mw = t1 - t0
                        if units[i][4] == 0 and n == 0 and hh == 0:
                            ob_t = oe_pool.tile([128, HID], BF, tag="ob")
                            dstate["ob"] = ob_t
                        if hh == 0:
                            po_t = psD.tile([128, 512], F32, tag=f"po{n % 2}")
                            dstate["po"] = po_t
                        po, ob = dstate["po"], dstate["ob"]
                        nc.tensor.matmul(
                            po[:mw, :],
                            attn_c[t0 // 512][:, hh,
                                              t0 - 512 * (t0 // 512):
                                              t1 - 512 * (t0 // 512)],
                            wd_t[:, ei, nh, hh, 512 * n:512 * (n + 1)],
                            start=(hh == 0), stop=(hh == HPC - 1))
                        if hh == HPC - 1:
                            dst = ob[:mw, 2048 * nh + 512 * n:
                                     2048 * nh + 512 * (n + 1)]
                            if n % 2 == 0:
                                nc.scalar.activation(out=dst, in_=po[:mw, :],
                                                     func=AF.Copy, scale=1.0)
                            else:
                                nc.vector.tensor_copy(dst, po[:mw, :])
                            if nh == 1 and n == 3:
                                nc.sync.dma_start(out=out_d[t0:t1, :],
                                                  in_=ob[:mw, :])
                        dstate["gi"] = i + 1
                        return True

                    def dense_ready(cur_c):
                        i = dstate["gi"]
                        return i < len(units) and units[i][0] < cur_c

                    for c in range(4):
                        for h in range(HPC):
                            nj = 4 * c + 4
                            ap_ps = psP.tile([128, 512], F32, tag="ap")
                            pacc = pa_pool.tile([128, 512], F32R, tag="pacc")
                            for j in range(nj):
                                # diagonal blocks: only cols >= 128r survive
                                r = j - 4 * c
                                x0 = 128 * r if r > 0 else 0
                                scp = psS.tile([128, 512], F32, tag="sc")
                                nc.tensor.matmul(
                                    scp[:, x0:],
                                    qk[:, 4 + h, 128 * j:128 * (j + 1)],
                                    qk[:, h, 512 * c + x0:512 * (c + 1)],
                                    start=True, stop=True)
                                pb = pr_pool.tile([128, 512], BF, tag="pb")
                                nc.scalar.activation(out=pb[:, x0:],
                                                     in_=scp[:, x0:],
                                                     func=AF.Exp, scale=SCALE)
                                if r >= 0:
                                    nc.vector.tensor_mul(
                                        pb[:], pb[:],
                                        mask_t[:, r, :])
                                if j == 0:
                                    nc.gpsimd.tensor_copy(pacc[:], pb[:])
                                else:
                                    nc.gpsimd.tensor_add(pacc[:, x0:],
                                                         pacc[:, x0:],
                                                         pb[:, x0:])
                                if dense_ready(c):
                                    dense_step()
                                if dense_ready(c):
                                    dense_step()
                                nc.tensor.matmul(
                                    ap_ps[:, x0:],
                                    vsb[:, j, 128 * h:128 * (h + 1)],
                                    pb[:, x0:],
                                    start=(j == 0), stop=(j == nj - 1))
                            sp_ps = psU.tile([128, 512], F32, tag="sp")
                            nc.tensor.matmul(sp_ps[:], ones_t[:], pacc[:],
                                             start=True, stop=True)
                            rb = sm_pool.tile([128, 512], F32, tag="rb")
                            nc.vector.reciprocal_approx_fast(out=rb[:],
                                                             in_=sp_ps[:])
                            nc.vector.tensor_mul(
                                attn_c[c][:, h, :], ap_ps[:], rb[:])
                    while dense_step():
                        pass
    nc.compile()
    return nc


def _prep_inputs(inputs):
    hs = np.asarray(inputs["hidden_states"], np.float32)
    cos = np.asarray(inputs["cos"], np.float32)
    sin = np.asarray(inputs["sin"], np.float32)
    vi = np.asarray(inputs["vision_indices"]).ravel()
    li = np.asarray(inputs["language_indices"]).ravel()
    nv = vi.size
    assert nv == NV and np.array_equal(vi, np.arange(nv)) and \
        np.array_equal(li, np.arange(nv, S)), "unsupported index layout"

    # hs tiled per (chunk, kt-quarter): hsq[ci, qi, p, k8, t] =
    # hs[c0 + t, 128*(8*qi + k8) + p]
    hsT = hs.T.astype(BF_NP)
    hsq = np.zeros((4, 4, 128, 8, 576), BF_NP)
    for ci, (c0, c1, _e, side) in enumerate(CHUNKS):
        ww = (c1 - c0) + (64 if side else 0)
        blk = hsT[:, c0:c0 + ww].reshape(4, 8, 128, ww)
        hsq[ci, :, :, :, :ww] = blk.transpose(0, 2, 1, 3)
    cosT = np.ascontiguousarray(cos.T).astype(BF_NP)
    sinT = np.ascontiguousarray(sin.T).astype(BF_NP)
    rmT = np.zeros((D, D), np.float32)
    for d in range(64):
        rmT[d + 64, d] = -1.0
        rmT[d, d + 64] = 1.0
    masks = np.zeros((128, 4, 512), np.float32)
    tri = np.tril(np.ones((128, 128), np.float32)).T  # [t, s]: 1 iff t <= s
    for r in range(4):
        blk = np.ones((128, 512), np.float32)
        blk[:, :128 * r] = 0.0
        blk[:, 128 * r:128 * (r + 1)] = tri
        masks[:, r, :] = blk
    b = np.asarray(inputs["vision_qkv_b"], np.float32)
    wq_all = {"V": np.asarray(inputs["vision_qkv_w"], np.float32),
              "L": np.asarray(inputs["lang_qkv_w"], np.float32)}
    wd_all = {"V": np.asarray(inputs["vision_dense_w"], np.float32),
              "L": np.asarray(inputs["lang_dense_w"], np.float32)}

    def qk_cols(W, r):
        cols = []
        for m in range(8):
            col0 = (0 if m < 4 else HID) + VC * r + 128 * (m % 4)
            cols.append(W[:, col0:col0 + 128])
        return np.stack(cols, 0)                   # [8, HID, 128]

    def qk_tiles(W, r):
        # stationary layout [8, 128, NKT, 128]
        arr = qk_cols(W, r)
        return np.ascontiguousarray(
            arr.reshape(8, NKT, 128, 128).transpose(0, 2, 1, 3)).astype(BF_NP)

    def qm_tiles(W, r):
        # moving layout for the side tokens: [8 ktc, 128, 4 k8, 1024]
        arr = qk_cols(W, r)                        # [8, HID, 128]
        arr = arr.transpose(1, 0, 2).reshape(HID, 1024)   # [HID, 8*128]
        return np.ascontiguousarray(
            arr.reshape(8, 4, 128, 1024).transpose(0, 2, 1, 3)).astype(BF_NP)

    def v_tiles(W, r):
        # [128, NKT, VC]
        c0 = 2 * HID + VC * r
        return np.ascontiguousarray(
            W[:, c0:c0 + VC].reshape(NKT, 128, VC).transpose(1, 0, 2)
        ).astype(BF_NP)

    def d_tiles(Wv, Wl, r):
        # [2(expert), 128, 2(nh), HPC, 2048]
        out = np.empty((2, 128, 2, HPC, 2048), np.float32)
        for ei, W in enumerate((Wv, Wl)):
            rows = W[VC * r:VC * r + VC, :]        # [512, 4096]
            blk = rows.reshape(HPC, 128, 2, 2048)  # [hh, p, nh, c]
            out[ei] = blk.transpose(1, 2, 0, 3)
        return np.ascontiguousarray(out).astype(BF_NP)

    in_maps = []
    for r in range(NCORES):
        bqk_r = np.empty((128, 8), np.float32)
        for m in range(8):
            col0 = (0 if m < 4 else HID) + VC * r + 128 * (m % 4)
            bqk_r[:, m] = b[col0:col0 + 128]
        in_maps.append({
            "hsq": hsq,
            "wqk_v": qk_tiles(wq_all["V"], r),
            "wqk_l": qk_tiles(wq_all["L"], r),
            "wv_v": v_tiles(wq_all["V"], r),
            "wv_l": v_tiles(wq_all["L"], r),
            "wqm": qm_tiles(wq_all["V"], r),
            "wdw": d_tiles(wd_all["V"], wd_all["L"], r),
            "bqk": bqk_r,
            "bv": np.ascontiguousarray(
                b[2 * HID + VC * r:2 * HID + VC * r + VC].reshape(1, VC)),
            "cosw": cosT, "sinw": sinT,
            "rmT": rmT.astype(BF_NP),
            "idm": np.eye(64, dtype=BF_NP),
            "ones": np.ones((128, 128), BF_NP),
            "ones_r": np.ones((128, 128), np.float32),
            "masks": masks.astype(BF_NP),
        })
    return in_maps


def kernel(**inputs):
    if "nc" not in _CACHE:
        _CACHE["nc"] = _build()
    nc = _CACHE["nc"]
    in_maps = _prep_inputs(inputs)
    res = run_bass_kernel_spmd(nc, in_maps, list(range(NCORES)),
                               **_CACHE.get("run_kwargs", {}))
    _CACHE["last_results"] = res
    out = np.zeros((S, HID), np.float64)
    for r in range(NCORES):
        out += res.results[r]["out"].astype(np.float64)
    return out.astype(np.float32)


# revision 25
# speedup vs baseline: 1.0294x; 1.0294x over previous
"""Trainium2 Bass kernel for modality-routed (CogVLM-style) attention.

Contract: kernel(**inputs) takes FULL unsharded numpy inputs (as produced by
the reference's setup_inputs) and returns the FULL [2048, 4096] fp32 output.

Sharding: tensor-parallel over heads. Core r owns heads 4r..4r+3:
  - qkv weights column-sharded; q/k computed weight-stationary producing
    qT/kT [d, tok] directly, v computed token-stationary in [tok, d].
  - dense weights row-sharded [512, 4096]; each core emits a partial
    [2048, 4096] fp32 output, summed on the host (the unshard step).

All PE work in bf16 (fp32 PSUM accumulation; bf16 output partials summed
in fp64 on the host). All intermediates (qT/kT, v, attn) stay SBUF-resident.
Host pretiles every tensor into its exact SBUF layout so each weight/hst
load is one contiguous-per-partition descriptor on a hardware-DGE queue
(sync/scalar; gpsimd software-DGE is ~50 GB/s and only runs softmax-sum
accumulation). Softmax: no max-subtraction (scores are O(10)); row sums
via gpsimd-accumulated probs + one all-ones fp32r matmul per (h, c-chunk)
whose PE output broadcasts the sums to all partitions; reciprocal via the
fast-approx DVE op; diagonal causal blocks compute only the surviving
columns. The 64 vision side tokens (512..576) run through moving-operand
matmuls + a PE transpose instead of 288 tiny 64-wide matmuls. Dense-phase
matmuls are interleaved one-by-one into attention's exp-wait bubbles
(per-chunk attn tiles avoid false cross-phase dependencies).
"""

import math
import sys

import numpy as np

if "/opt/trn_rl_repo" not in sys.path:
    sys.path.insert(0, "/opt/trn_rl_repo")

import ml_dtypes  # noqa: E402

import concourse.bass as bass  # noqa: E402,F401
import concourse.tile as tile  # noqa: E402
from concourse import bacc, mybir  # noqa: E402
from concourse.bass_utils import run_bass_kernel_spmd  # noqa: E402

S = 2048
HID = 4096
H = 32
D = 128
NCORES = 8
HPC = H // NCORES          # heads per core = 4
VC = HPC * D               # per-core q (or k or v) width = 512
NV = 576                   # vision tokens occupy rows [0, NV)
NKT = HID // 128           # 32 contraction tiles
SCALE = 1.0 / math.sqrt(D)

BF = mybir.dt.bfloat16
F32 = mybir.dt.float32
F32R = mybir.dt.float32r
BF_NP = ml_dtypes.bfloat16
AF = mybir.ActivationFunctionType

_CACHE = {}

CHUNKS = [(0, 512, "V", True), (NV, 1024, "L", False),
          (1024, 1536, "L", False), (1536, 2048, "L", False)]


def _token_tiles(t0, t1):
    out = []
    c = t0
    while c < t1:
        n = min(t1, (c // 128 + 1) * 128)
        out.append((c, n))
        c = n
    return out


def _build():
    nc = bacc.Bacc("TRN2", target_bir_lowering=False, debug=False,
                   num_devices=NCORES)
    dti = nc.dram_tensor
    hsq = dti("hsq", [4, 4, 128, 8, 576], BF, kind="ExternalInput").ap()
    wqk_v = dti("wqk_v", [8, 128, NKT, 128], BF, kind="ExternalInput").ap()
    wqk_l = dti("wqk_l", [8, 128, NKT, 128], BF, kind="ExternalInput").ap()
    wv_v = dti("wv_v", [128, NKT, VC], BF, kind="ExternalInput").ap()
    wv_l = dti("wv_l", [128, NKT, VC], BF, kind="ExternalInput").ap()
    wqm = dti("wqm", [8, 128, 4, 1024], BF, kind="ExternalInput").ap()
    wdw = dti("wdw", [2, 128, 2, HPC, 2048], BF, kind="ExternalInput").ap()
    bqk = dti("bqk", [128, 8], F32, kind="ExternalInput").ap()
    bv = dti("bv", [1, VC], F32, kind="ExternalInput").ap()
    cosw = dti("cosw", [D, S], BF, kind="ExternalInput").ap()
    sinw = dti("sinw", [D, S], BF, kind="ExternalInput").ap()
    rmT = dti("rmT", [D, D], BF, kind="ExternalInput").ap()
    idm = dti("idm", [64, 64], BF, kind="ExternalInput").ap()
    ones = dti("ones", [128, 128], BF, kind="ExternalInput").ap()
    ones_r = dti("ones_r", [128, 128], F32R, kind="ExternalInput").ap()
    masks = dti("masks", [128, 4, 512], BF, kind="ExternalInput").ap()
    out_d = dti("out", [S, HID], BF, kind="ExternalOutput").ap()

    with tile.TileContext(nc) as tc:
        with tc.tile_pool(name="glob", bufs=1) as glob:
            qk = glob.tile([128, 8, S], BF)        # qT (m 0..3) / kT (m 4..7)
            vsb = glob.tile([128, 16, VC], BF)     # v[128t+p, :] token tiles

            # ---------------- QKV phase ----------------
            with tc.tile_pool(name="hsA", bufs=2) as hsA_pool, \
                 tc.tile_pool(name="hsB", bufs=2) as hsB_pool, \
                 tc.tile_pool(name="hsC", bufs=2) as hsC_pool, \
                 tc.tile_pool(name="hsD", bufs=2) as hsD_pool, \
                 tc.tile_pool(name="cns", bufs=1) as cns, \
                 tc.tile_pool(name="wq", bufs=2) as wq_pool, \
                 tc.tile_pool(name="wvp", bufs=1) as wv_pool, \
                 tc.tile_pool(name="wqmp", bufs=2) as wqm_pool, \
                 tc.tile_pool(name="ev", bufs=2) as ev_pool, \
                 tc.tile_pool(name="psA", bufs=2, space="PSUM") as psA, \
                 tc.tile_pool(name="psR", bufs=2, space="PSUM") as psR, \
                 tc.tile_pool(name="psSd", bufs=1, space="PSUM") as psSd, \
                 tc.tile_pool(name="psT", bufs=1, space="PSUM") as psT:
                # first chunk's activations + weights land first (parallel
                # queues); consts follow on the vector queue.
                hs_tiles = []

                def load_hst(ci):
                    (c0, c1, _e, side) = CHUNKS[ci]
                    ww = (c1 - c0) + (64 if side else 0)
                    ha = hsA_pool.tile([128, 8, 576], BF, tag="hsA")
                    hb = hsB_pool.tile([128, 8, 576], BF, tag="hsB")
                    hc = hsC_pool.tile([128, 8, 576], BF, tag="hsC")
                    hd = hsD_pool.tile([128, 8, 576], BF, tag="hsD")
                    for qi, hq in enumerate((ha, hb, hc, hd)):
                        nc.sync.dma_start(out=hq[:, :, :ww],
                                          in_=hsq[ci, qi, :, :, :ww])
                    hs_tiles.append((ha, hb, hc, hd))

                for ci, (c0, c1, e, side) in enumerate(CHUNKS[:1]):
                    wqk = wqk_v if e == "V" else wqk_l
                    if ci == 0:
                        wt0 = wq_pool.tile([128, NKT, 128], BF, tag="wt")
                        nc.sync.dma_start(out=wt0[:, 0:8, :],
                                          in_=wqk[0][:, 0:8, :])
                        load_hst(0)
                        nc.sync.dma_start(out=wt0[:, 8:32, :],
                                          in_=wqk[0][:, 8:32, :])
                        load_hst(1)
                        cos_t = cns.tile([D, S], BF)
                        nc.scalar.dma_start(out=cos_t[:], in_=cosw[:])
                        sin_t = cns.tile([D, S], BF)
                        nc.scalar.dma_start(out=sin_t[:], in_=sinw[:])
                        rm_t = cns.tile([D, D], BF)
                        nc.scalar.dma_start(out=rm_t[:], in_=rmT[:])
                        idm_t = cns.tile([64, 64], BF)
                        nc.scalar.dma_start(out=idm_t[:], in_=idm[:])
                        bqk_t = cns.tile([128, 8], F32)
                        nc.scalar.dma_start(out=bqk_t[:], in_=bqk[:])
                        bv_t = cns.tile([128, VC], F32)
                        nc.scalar.dma_start(out=bv_t[:],
                                            in_=bv[:].to_broadcast((128, VC)))

                def hsrc(ci, kt):
                    return hs_tiles[ci][kt // 8], kt % 8

                for ci, (c0, c1, e, side) in enumerate(CHUNKS):
                    w = c1 - c0
                    wqk = wqk_v if e == "V" else wqk_l
                    wv = wv_v if e == "V" else wv_l
                    if ci + 2 < len(CHUNKS):
                        load_hst(ci + 2)
                    # --- q/k, weight-stationary -> qT/kT [d, tok] + RoPE ---
                    wvt = None
                    for m in range(8):
                        if ci == 0 and m == 0:
                            wt = wt0
                        else:
                            wt = wq_pool.tile([128, NKT, 128], BF, tag="wt")
                            nc.sync.dma_start(out=wt[:], in_=wqk[m])
                        if m == 3:
                            # v weights: prefetch mid m-loop (prior chunk's v
                            # reads are long done -> no queue stall)
                            wvt = wv_pool.tile([128, NKT, VC], BF, tag="wvt")
                            nc.scalar.dma_start(out=wvt[:], in_=wv[:])
                        pt = psA.tile([128, 512], F32, tag="pt")
                        for kt in range(NKT):
                            hs_t, k2 = hsrc(ci, kt)
                            nc.tensor.matmul(pt[:, :w], wt[:, kt, :],
                                             hs_t[:, k2, :w],
                                             start=(kt == 0),
                                             stop=(kt == NKT - 1))
                        qs = ev_pool.tile([128, 512], BF, tag="qs")
                        if e == "V":
                            nc.scalar.activation(out=qs[:, :w], in_=pt[:, :w],
                                                 func=AF.Identity,
                                                 bias=bqk_t[:, m:m + 1],
                                                 scale=1.0)
                        else:
                            nc.scalar.activation(out=qs[:, :w], in_=pt[:, :w],
                                                 func=AF.Copy, scale=1.0)
                        rot = psR.tile([128, 512], F32, tag="rot")
                        nc.tensor.matmul(rot[:, :w], rm_t[:], qs[:, :w],
                                         start=True, stop=True)
                        tb = ev_pool.tile([128, 512], BF, tag="tb")
                        nc.vector.tensor_mul(tb[:, :w], rot[:, :w],
                                             sin_t[:, c0:c1])
                        qc = ev_pool.tile([128, 512], BF, tag="qc")
                        nc.vector.tensor_mul(qc[:, :w], qs[:, :w],
                                             cos_t[:, c0:c1])
                        nc.vector.tensor_add(qk[:, m, c0:c1], qc[:, :w],
                                             tb[:, :w])
                    # --- v, token-stationary -> v [tok, d] ---
                    for (t0, t1) in _token_tiles(c0, c1):
                        mw = t1 - t0
                        pv = psA.tile([128, 512], F32, tag="pt")
                        for kt in range(NKT):
                            hs_t, k2 = hsrc(ci, kt)
                            nc.tensor.matmul(
                                pv[:mw, :], hs_t[:, k2, t0 - c0:t1 - c0],
                                wvt[:, kt, :],
                                start=(kt == 0), stop=(kt == NKT - 1))
                        tt, po = t0 // 128, t0 % 128
                        if po == 0:
                            if e == "V":
                                nc.vector.tensor_add(vsb[:mw, tt, :],
                                                     pv[:mw, :], bv_t[:mw, :])
                            else:
                                nc.scalar.activation(out=vsb[:mw, tt, :],
                                                     in_=pv[:mw, :],
                                                     func=AF.Copy, scale=1.0)
                        else:
                            # tokens 576..640: partition-offset fixup via DMA
                            vs = ev_pool.tile([128, 512], BF, tag="vs")
                            nc.scalar.activation(out=vs[:mw, :],
                                                 in_=pv[:mw, :],
                                                 func=AF.Copy, scale=1.0)
                            nc.gpsimd.dma_start(out=vsb[po:po + mw, tt, :],
                                                in_=vs[:mw, :])
                    # --- vision side tokens 512..576: moving-operand qkv ---
                    if side:
                        psd = []
                        for p in range(3):
                            psd_p = psSd.tile([128, 512], F32, tag=f"psd{p}")
                            psd.append(psd_p)
                        for ktc in range(8):
                            wq_c = wqm_pool.tile([128, 4, 1024], BF,
                                                 tag="wq_c")
                            nc.sync.dma_start(out=wq_c[:], in_=wqm[ktc])
                            for k8 in range(4):
                                kt = 4 * ktc + k8
                                hs_t, k2 = hsrc(ci, kt)
                                st = hs_t[:, k2, 512:576]
                                nc.tensor.matmul(
                                    psd[0][:64, :], st, wq_c[:, k8, 0:512],
                                    start=(kt == 0), stop=(kt == NKT - 1))
                                nc.tensor.matmul(
                                    psd[1][:64, :], st, wq_c[:, k8, 512:1024],
                                    start=(kt == 0), stop=(kt == NKT - 1))
                                wv_mv = wvt[:, kt, :]
                                nc.tensor.matmul(
                                    psd[2][:64, :], st, wv_mv,
                                    start=(kt == 0), stop=(kt == NKT - 1))
                        # v side: bias along free dim, straight into vsb
                        nc.vector.tensor_add(vsb[:64, 4, :], psd[2][:64, :],
                                             bv_t[:64, :])
                        # q/k side: evac, transpose to [d, tok], rope
                        sqk = ev_pool.tile([128, 8, 128], BF, tag="sqk")
                        nc.scalar.activation(out=sqk[:64, 0:4, :],
                                             in_=psd[0][:64, :],
                                             func=AF.Copy, scale=1.0)
                        nc.scalar.activation(out=sqk[:64, 4:8, :],
                                             in_=psd[1][:64, :],
                                             func=AF.Copy, scale=1.0)
                        for m in range(8):
                            tp = psT.tile([128, 64], BF, tag="tp")
                            nc.tensor.transpose(tp[:], sqk[:64, m, :],
                                                idm_t[:])
                            qs2 = ev_pool.tile([128, 64], BF, tag="qs2")
                            nc.scalar.activation(out=qs2[:], in_=tp[:],
                                                 func=AF.Identity,
                                                 bias=bqk_t[:, m:m + 1],
                                                 scale=1.0)
                            rot2 = psR.tile([128, 512], F32, tag="rot")
                            nc.tensor.matmul(rot2[:, :64], rm_t[:], qs2[:],
                                             start=True, stop=True)
                            tb2 = ev_pool.tile([128, 64], BF, tag="tb2")
                            nc.vector.tensor_mul(tb2[:], rot2[:, :64],
                                                 sin_t[:, 512:NV])
                            qc2 = ev_pool.tile([128, 64], BF, tag="qc2")
                            nc.vector.tensor_mul(qc2[:], qs2[:],
                                                 cos_t[:, 512:NV])
                            nc.vector.tensor_add(qk[:, m, 512:NV], qc2[:],
                                                 tb2[:])

            # ------------- attention + dense phases -------------
            with tc.tile_pool(name="wd", bufs=1) as wd_pool, \
                 tc.tile_pool(name="an", bufs=1) as an_pool:
                # prefetch dense weights during attention
                wd_t = wd_pool.tile([128, 2, 2, HPC, 2048], BF, tag="wd")
                nc.scalar.dma_start(out=wd_t[:, 0], in_=wdw[0])
                nc.scalar.dma_start(out=wd_t[:, 1], in_=wdw[1])
                attn_c = []
                for cc in range(4):
                    attn_t = an_pool.tile([128, HPC, 512], BF, tag=f"at{cc}")
                    attn_c.append(attn_t)
                with tc.tile_pool(name="acns", bufs=1) as acns, \
                     tc.tile_pool(name="pr", bufs=3) as pr_pool, \
                     tc.tile_pool(name="pa", bufs=2) as pa_pool, \
                     tc.tile_pool(name="sm", bufs=2) as sm_pool, \
                     tc.tile_pool(name="oe", bufs=3) as oe_pool, \
                     tc.tile_pool(name="psS", bufs=3, space="PSUM") as psS, \
                     tc.tile_pool(name="psP", bufs=2, space="PSUM") as psP, \
                     tc.tile_pool(name="psU", bufs=1, space="PSUM") as psU, \
                     tc.tile_pool(name="psD", bufs=1, space="PSUM") as psD:
                    mask_t = acns.tile([128, 4, 512], BF)
                    nc.scalar.dma_start(out=mask_t[:], in_=masks[:])
                    ones_t = acns.tile([128, 128], F32R)
                    nc.scalar.dma_start(out=ones_t[:], in_=ones_r[:])
                    for _ in range(3):
                        pb0 = pr_pool.tile([128, 512], BF, tag="pb")
                        nc.vector.memset(pb0[:], 0.0)

                    # dense work: one unit = one hh-matmul of a po group
                    # ((range, nh, n) accumulated over hh); interleaved into
                    # attention's exp-wait bubbles once the needed attn
                    # token-chunk is complete.
                    ranges = []
                    for (t0, t1) in _token_tiles(0, S):
                        if t0 < NV < t1:
                            ranges.append((t0, NV, 0))
                            ranges.append((NV, t1, 1))
                        else:
                            ranges.append((t0, t1, 0 if t0 < NV else 1))
                    units = []
                    for (t0, t1, ei) in ranges:
                        for nh in range(2):
                            for n in range(4):
                                for hh in range(HPC):
                                    units.append((t0 // 512, t0, t1, ei,
                                                  nh, n, hh))
                    dstate = {"gi": 0, "po": None, "ob": None}

                    def dense_step():
                        i = dstate["gi"]
                        if i >= len(units):
                            return False
                        (_, t0, t1, ei, nh, n, hh) = units[i]
                        mw = t1 - t0
                        if units[i][4] == 0 and n == 0 and hh == 0:
                            ob_t = oe_pool.tile([128, HID], BF, tag="ob")
                            dstate["ob"] = ob_t
                        if hh == 0:
                            po_t = psD.tile([128, 512], F32, tag=f"po{n % 2}")
                            dstate["po"] = po_t
                        po, ob = dstate["po"], dstate["ob"]
                        nc.tensor.matmul(
                            po[:mw, :],
                            attn_c[t0 // 512][:, hh,
                                              t0 - 512 * (t0 // 512):
                                              t1 - 512 * (t0 // 512)],
                            wd_t[:, ei, nh, hh, 512 * n:512 * (n + 1)],
                            start=(hh == 0), stop=(hh == HPC - 1))
                        if hh == HPC - 1:
                            dst = ob[:mw, 2048 * nh + 512 * n:
                                     2048 * nh + 512 * (n + 1)]
                            if n % 2 == 0:
                                nc.scalar.activation(out=dst, in_=po[:mw, :],
                                                     func=AF.Copy, scale=1.0)
                            else:
                                nc.vector.tensor_copy(dst, po[:mw, :])
                            if nh == 1 and n == 3:
                                nc.sync.dma_start(out=out_d[t0:t1, :],
                                                  in_=ob[:mw, :])
                        dstate["gi"] = i + 1
                        return True

                    def dense_ready(cur_c):
                        i = dstate["gi"]
                        return i < len(units) and units[i][0] < cur_c

                    for c in range(4):
                        for h in range(HPC):
                            nj = 4 * c + 4
                            ap_ps = psP.tile([128, 512], F32, tag="ap")
                            pacc = pa_pool.tile([128, 512], F32R, tag="pacc")
                            pacv = pa_pool.tile([128, 512], F32R, tag="pacv")
                            for j in range(nj):
                                # diagonal blocks: only cols >= 128r survive
                                r = j - 4 * c
                                x0 = 128 * r if r > 0 else 0
                                scp = psS.tile([128, 512], F32, tag="sc")
                                nc.tensor.matmul(
                                    scp[:, x0:],
                                    qk[:, 4 + h, 128 * j:128 * (j + 1)],
                                    qk[:, h, 512 * c + x0:512 * (c + 1)],
                                    start=True, stop=True)
                                pb = pr_pool.tile([128, 512], BF, tag="pb")
                                nc.scalar.activation(out=pb[:, x0:],
                                                     in_=scp[:, x0:],
                                                     func=AF.Exp, scale=SCALE)
                                if r >= 0:
                                    nc.vector.tensor_mul(
                                        pb[:], pb[:],
                                        mask_t[:, r, :])
                                eng = nc.gpsimd if j % 2 == 0 else nc.vector
                                pac = pacc if j % 2 == 0 else pacv
                                if j < 2:
                                    eng.tensor_copy(pac[:], pb[:])
                                else:
                                    eng.tensor_add(pac[:, x0:], pac[:, x0:],
                                                   pb[:, x0:])
                                if dense_ready(c):
                                    dense_step()
                                if dense_ready(c):
                                    dense_step()
                                nc.tensor.matmul(
                                    ap_ps[:, x0:],
                                    vsb[:, j, 128 * h:128 * (h + 1)],
                                    pb[:, x0:],
                                    start=(j == 0), stop=(j == nj - 1))
                            sp_ps = psU.tile([128, 512], F32, tag="sp")
                            nc.tensor.matmul(sp_ps[:], ones_t[:], pacc[:],
                                             start=True, stop=False)
                            nc.tensor.matmul(sp_ps[:], ones_t[:], pacv[:],
                                             start=False, stop=True)
                            rb = sm_pool.tile([128, 512], F32, tag="rb")
                            nc.vector.reciprocal_approx_fast(out=rb[:],
                                                             in_=sp_ps[:])
                            nc.vector.tensor_mul(
                                attn_c[c][:, h, :], ap_ps[:], rb[:])
                    while dense_step():
                        pass
    nc.compile()
    return nc


def _prep_inputs(inputs):
    hs = np.asarray(inputs["hidden_states"], np.float32)
    cos = np.asarray(inputs["cos"], np.float32)
    sin = np.asarray(inputs["sin"], np.float32)
    vi = np.asarray(inputs["vision_indices"]).ravel()
    li = np.asarray(inputs["language_indices"]).ravel()
    nv = vi.size
    assert nv == NV and np.array_equal(vi, np.arange(nv)) and \
        np.array_equal(li, np.arange(nv, S)), "unsupported index layout"

    # hs tiled per (chunk, kt-quarter): hsq[ci, qi, p, k8, t] =
    # hs[c0 + t, 128*(8*qi + k8) + p]
    hsT = hs.T.astype(BF_NP)
    hsq = np.zeros((4, 4, 128, 8, 576), BF_NP)
    for ci, (c0, c1, _e, side) in enumerate(CHUNKS):
        ww = (c1 - c0) + (64 if side else 0)
        blk = hsT[:, c0:c0 + ww].reshape(4, 8, 128, ww)
        hsq[ci, :, :, :, :ww] = blk.transpose(0, 2, 1, 3)
    cosT = np.ascontiguousarray(cos.T).astype(BF_NP)
    sinT = np.ascontiguousarray(sin.T).astype(BF_NP)
    rmT = np.zeros((D, D), np.float32)
    for d in range(64):
        rmT[d + 64, d] = -1.0
        rmT[d, d + 64] = 1.0
    masks = np.zeros((128, 4, 512), np.float32)
    tri = np.tril(np.ones((128, 128), np.float32)).T  # [t, s]: 1 iff t <= s
    for r in range(4):
        blk = np.ones((128, 512), np.float32)
        blk[:, :128 * r] = 0.0
        blk[:, 128 * r:128 * (r + 1)] = tri
        masks[:, r, :] = blk
    b = np.asarray(inputs["vision_qkv_b"], np.float32)
    wq_all = {"V": np.asarray(inputs["vision_qkv_w"], np.float32),
              "L": np.asarray(inputs["lang_qkv_w"], np.float32)}
    wd_all = {"V": np.asarray(inputs["vision_dense_w"], np.float32),
              "L": np.asarray(inputs["lang_dense_w"], np.float32)}

    def qk_cols(W, r):
        cols = []
        for m in range(8):
            col0 = (0 if m < 4 else HID) + VC * r + 128 * (m % 4)
            cols.append(W[:, col0:col0 + 128])
        return np.stack(cols, 0)                   # [8, HID, 128]

    def qk_tiles(W, r):
        # stationary layout [8, 128, NKT, 128]
        arr = qk_cols(W, r)
        return np.ascontiguousarray(
            arr.reshape(8, NKT, 128, 128).transpose(0, 2, 1, 3)).astype(BF_NP)

    def qm_tiles(W, r):
        # moving layout for the side tokens: [8 ktc, 128, 4 k8, 1024]
        arr = qk_cols(W, r)                        # [8, HID, 128]
        arr = arr.transpose(1, 0, 2).reshape(HID, 1024)   # [HID, 8*128]
        return np.ascontiguousarray(
            arr.reshape(8, 4, 128, 1024).transpose(0, 2, 1, 3)).astype(BF_NP)

    def v_tiles(W, r):
        # [128, NKT, VC]
        c0 = 2 * HID + VC * r
        return np.ascontiguousarray(
            W[:, c0:c0 + VC].reshape(NKT, 128, VC).transpose(1, 0, 2)
        ).astype(BF_NP)

    def d_tiles(Wv, Wl, r):
        # [2(expert), 128, 2(nh), HPC, 2048]
        out = np.empty((2, 128, 2, HPC, 2048), np.float32)
        for ei, W in enumerate((Wv, Wl)):
            rows = W[VC * r:VC * r + VC, :]        # [512, 4096]
            blk = rows.reshape(HPC, 128, 2, 2048)  # [hh, p, nh, c]
            out[ei] = blk.transpose(1, 2, 0, 3)
        return np.ascontiguousarray(out).astype(BF_NP)

    in_maps = []
    for r in range(NCORES):
        bqk_r = np.empty((128, 8), np.float32)
        for m in range(8):
            col0 = (0 if m < 4 else HID) + VC * r + 128 * (m % 4)
            bqk_r[:, m] = b[col0:col0 + 128]
        in_maps.append({
            "hsq": hsq,
            "wqk_v": qk_tiles(wq_all["V"], r),
            "wqk_l": qk_tiles(wq_all["L"], r),
            "wv_v": v_tiles(wq_all["V"], r),
            "wv_l": v_tiles(wq_all["L"], r),
            "wqm": qm_tiles(wq_all["V"], r),
            "wdw": d_tiles(wd_all["V"], wd_all["L"], r),
            "bqk": bqk_r,
            "bv": np.ascontiguousarray(
                b[2 * HID + VC * r:2 * HID + VC * r + VC].reshape(1, VC)),
            "cosw": cosT, "sinw": sinT,
            "rmT": rmT.astype(BF_NP),
            "idm": np.eye(64, dtype=BF_NP),
            "ones": np.ones((128, 128), BF_NP),
            "ones_r": np.ones((128, 128), np.float32),
            "masks": masks.astype(BF_NP),
        })
    return in_maps


def kernel(**inputs):
    if "nc" not in _CACHE:
        _CACHE["nc"] = _build()
    nc = _CACHE["nc"]
    in_maps = _prep_inputs(inputs)
    res = run_bass_kernel_spmd(nc, in_maps, list(range(NCORES)),
                               **_CACHE.get("run_kwargs", {}))
    _CACHE["last_results"] = res
    out = np.zeros((S, HID), np.float64)
    for r in range(NCORES):
        out += res.results[r]["out"].astype(np.float64)
    return out.astype(np.float32)


# revision 26
# speedup vs baseline: 1.2228x; 1.1878x over previous
"""Trainium2 Bass kernel for modality-routed (CogVLM-style) attention.

Contract: kernel(**inputs) takes FULL unsharded numpy inputs (as produced by
the reference's setup_inputs) and returns the FULL [2048, 4096] fp32 output.

Sharding: tensor-parallel over heads. Core r owns heads 4r..4r+3:
  - qkv weights column-sharded; q/k computed weight-stationary producing
    qT/kT [d, tok] directly, v computed token-stationary in [tok, d].
  - dense weights row-sharded [512, 4096]; each core emits a partial
    [2048, 4096] fp32 output, summed on the host (the unshard step).

All PE work in bf16 (fp32 PSUM accumulation; bf16 output partials summed
in fp64 on the host). All intermediates (qT/kT, v, attn) stay SBUF-resident.
Host pretiles every tensor into its exact SBUF layout so each weight/hst
load is one contiguous-per-partition descriptor on a hardware-DGE queue
(sync/scalar; gpsimd software-DGE is ~50 GB/s and only runs softmax-sum
accumulation). Softmax: no max-subtraction (scores are O(10)); row sums
via gpsimd-accumulated probs + one all-ones fp32r matmul per (h, c-chunk)
whose PE output broadcasts the sums to all partitions; reciprocal via the
fast-approx DVE op; diagonal causal blocks compute only the surviving
columns. The 64 vision side tokens (512..576) run through moving-operand
matmuls + a PE transpose instead of 288 tiny 64-wide matmuls. Dense-phase
matmuls are interleaved one-by-one into attention's exp-wait bubbles
(per-chunk attn tiles avoid false cross-phase dependencies).
"""

import math
import sys

import numpy as np

if "/opt/trn_rl_repo" not in sys.path:
    sys.path.insert(0, "/opt/trn_rl_repo")

import ml_dtypes  # noqa: E402

import concourse.bass as bass  # noqa: E402,F401
import concourse.tile as tile  # noqa: E402
from concourse import bacc, mybir  # noqa: E402
from concourse.bass_utils import run_bass_kernel_spmd  # noqa: E402

S = 2048
HID = 4096
H = 32
D = 128
NCORES = 8
HPC = H // NCORES          # heads per core = 4
VC = HPC * D               # per-core q (or k or v) width = 512
NV = 576                   # vision tokens occupy rows [0, NV)
NKT = HID // 128           # 32 contraction tiles
SCALE = 1.0 / math.sqrt(D)

BF = mybir.dt.bfloat16
F32 = mybir.dt.float32
F32R = mybir.dt.float32r
BF_NP = ml_dtypes.bfloat16
AF = mybir.ActivationFunctionType

_CACHE = {}

CHUNKS = [(0, 512, "V", True), (NV, 1024, "L", False),
          (1024, 1536, "L", False), (1536, 2048, "L", False)]


def _token_tiles(t0, t1):
    out = []
    c = t0
    while c < t1:
        n = min(t1, (c // 128 + 1) * 128)
        out.append((c, n))
        c = n
    return out


def _build():
    nc = bacc.Bacc("TRN2", target_bir_lowering=False, debug=False,
                   num_devices=NCORES)
    dti = nc.dram_tensor
    hsq = dti("hsq", [4, 4, 128, 8, 576], BF, kind="ExternalInput").ap()
    wqk_v = dti("wqk_v", [8, 128, NKT, 128], BF, kind="ExternalInput").ap()
    wqk_l = dti("wqk_l", [8, 128, NKT, 128], BF, kind="ExternalInput").ap()
    wv_v = dti("wv_v", [128, NKT, VC], BF, kind="ExternalInput").ap()
    wv_l = dti("wv_l", [128, NKT, VC], BF, kind="ExternalInput").ap()
    wqm = dti("wqm", [8, 128, 4, 1024], BF, kind="ExternalInput").ap()
    wdw = dti("wdw", [2, 128, 2, HPC, 2048], BF, kind="ExternalInput").ap()
    bqk = dti("bqk", [128, 8], F32, kind="ExternalInput").ap()
    bv = dti("bv", [1, VC], F32, kind="ExternalInput").ap()
    cosw = dti("cosw", [D, S], BF, kind="ExternalInput").ap()
    sinw = dti("sinw", [D, S], BF, kind="ExternalInput").ap()
    rmT = dti("rmT", [D, D], BF, kind="ExternalInput").ap()
    idm = dti("idm", [64, 64], BF, kind="ExternalInput").ap()
    ones = dti("ones", [128, 128], BF, kind="ExternalInput").ap()
    ones_r = dti("ones_r", [128, 128], F32R, kind="ExternalInput").ap()
    masks = dti("masks", [128, 4, 512], BF, kind="ExternalInput").ap()
    out_d = dti("out", [S, HID], BF, kind="ExternalOutput").ap()

    with tile.TileContext(nc) as tc:
        with tc.tile_pool(name="glob", bufs=1) as glob:
            qk = glob.tile([128, 8, S], BF)        # qT (m 0..3) / kT (m 4..7)
            vsb = glob.tile([128, 16, VC], BF)     # v[128t+p, :] token tiles

            # ---------------- QKV phase ----------------
            with tc.tile_pool(name="hsA", bufs=2) as hsA_pool, \
                 tc.tile_pool(name="hsB", bufs=2) as hsB_pool, \
                 tc.tile_pool(name="hsC", bufs=2) as hsC_pool, \
                 tc.tile_pool(name="hsD", bufs=2) as hsD_pool, \
                 tc.tile_pool(name="cns", bufs=1) as cns, \
                 tc.tile_pool(name="wq", bufs=2) as wq_pool, \
                 tc.tile_pool(name="wvp", bufs=1) as wv_pool, \
                 tc.tile_pool(name="wqmp", bufs=2) as wqm_pool, \
                 tc.tile_pool(name="ev", bufs=2) as ev_pool, \
                 tc.tile_pool(name="psA", bufs=2, space="PSUM") as psA, \
                 tc.tile_pool(name="psR", bufs=2, space="PSUM") as psR, \
                 tc.tile_pool(name="psSd", bufs=1, space="PSUM") as psSd, \
                 tc.tile_pool(name="psT", bufs=1, space="PSUM") as psT:
                # first chunk's activations + weights land first (parallel
                # queues); consts follow on the vector queue.
                hs_tiles = []

                def load_hst(ci):
                    (c0, c1, _e, side) = CHUNKS[ci]
                    ww = (c1 - c0) + (64 if side else 0)
                    ha = hsA_pool.tile([128, 8, 576], BF, tag="hsA")
                    hb = hsB_pool.tile([128, 8, 576], BF, tag="hsB")
                    hc = hsC_pool.tile([128, 8, 576], BF, tag="hsC")
                    hd = hsD_pool.tile([128, 8, 576], BF, tag="hsD")
                    for qi, hq in enumerate((ha, hb, hc, hd)):
                        nc.sync.dma_start(out=hq[:, :, :ww],
                                          in_=hsq[ci, qi, :, :, :ww])
                    hs_tiles.append((ha, hb, hc, hd))

                for ci, (c0, c1, e, side) in enumerate(CHUNKS[:1]):
                    wqk = wqk_v if e == "V" else wqk_l
                    if ci == 0:
                        wt0 = wq_pool.tile([128, NKT, 128], BF, tag="wt")
                        nc.sync.dma_start(out=wt0[:, 0:8, :],
                                          in_=wqk[0][:, 0:8, :])
                        load_hst(0)
                        nc.sync.dma_start(out=wt0[:, 8:32, :],
                                          in_=wqk[0][:, 8:32, :])
                        load_hst(1)
                        cos_t = cns.tile([D, S], BF)
                        nc.scalar.dma_start(out=cos_t[:], in_=cosw[:])
                        sin_t = cns.tile([D, S], BF)
                        nc.scalar.dma_start(out=sin_t[:], in_=sinw[:])
                        rm_t = cns.tile([D, D], BF)
                        nc.scalar.dma_start(out=rm_t[:], in_=rmT[:])
                        idm_t = cns.tile([64, 64], BF)
                        nc.scalar.dma_start(out=idm_t[:], in_=idm[:])
                        bqk_t = cns.tile([128, 8], F32)
                        nc.scalar.dma_start(out=bqk_t[:], in_=bqk[:])
                        bv_t = cns.tile([128, VC], F32)
                        nc.scalar.dma_start(out=bv_t[:],
                                            in_=bv[:].to_broadcast((128, VC)))

                def hsrc(ci, kt):
                    return hs_tiles[ci][kt // 8], kt % 8

                for ci, (c0, c1, e, side) in enumerate(CHUNKS):
                    w = c1 - c0
                    wqk = wqk_v if e == "V" else wqk_l
                    wv = wv_v if e == "V" else wv_l
                    if ci + 2 < len(CHUNKS):
                        load_hst(ci + 2)
                    # --- q/k, weight-stationary -> qT/kT [d, tok] + RoPE ---
                    wvt = None
                    for m in range(8):
                        if ci == 0 and m == 0:
                            wt = wt0
                        else:
                            wt = wq_pool.tile([128, NKT, 128], BF, tag="wt")
                            nc.sync.dma_start(out=wt[:], in_=wqk[m])
                        if m == 3:
                            # v weights: prefetch mid m-loop (prior chunk's v
                            # reads are long done -> no queue stall)
                            wvt = wv_pool.tile([128, NKT, VC], BF, tag="wvt")
                            nc.scalar.dma_start(out=wvt[:], in_=wv[:])
                        pt = psA.tile([128, 512], F32, tag="pt")
                        for kt in range(NKT):
                            hs_t, k2 = hsrc(ci, kt)
                            nc.tensor.matmul(pt[:, :w], wt[:, kt, :],
                                             hs_t[:, k2, :w],
                                             start=(kt == 0),
                                             stop=(kt == NKT - 1))
                        qs = ev_pool.tile([128, 512], BF, tag="qs")
                        if e == "V":
                            nc.scalar.activation(out=qs[:, :w], in_=pt[:, :w],
                                                 func=AF.Identity,
                                                 bias=bqk_t[:, m:m + 1],
                                                 scale=1.0)
                        else:
                            nc.scalar.activation(out=qs[:, :w], in_=pt[:, :w],
                                                 func=AF.Copy, scale=1.0)
                        rot = psR.tile([128, 512], F32, tag="rot")
                        nc.tensor.matmul(rot[:, :w], rm_t[:], qs[:, :w],
                                         start=True, stop=True)
                        tb = ev_pool.tile([128, 512], BF, tag="tb")
                        nc.vector.tensor_mul(tb[:, :w], rot[:, :w],
                                             sin_t[:, c0:c1])
                        qc = ev_pool.tile([128, 512], BF, tag="qc")
                        nc.vector.tensor_mul(qc[:, :w], qs[:, :w],
                                             cos_t[:, c0:c1])
                        nc.vector.tensor_add(qk[:, m, c0:c1], qc[:, :w],
                                             tb[:, :w])
                    # --- v, token-stationary -> v [tok, d] ---
                    for (t0, t1) in _token_tiles(c0, c1):
                        mw = t1 - t0
                        pv = psA.tile([128, 512], F32, tag="pt")
                        for kt in range(NKT):
                            hs_t, k2 = hsrc(ci, kt)
                            nc.tensor.matmul(
                                pv[:mw, :], hs_t[:, k2, t0 - c0:t1 - c0],
                                wvt[:, kt, :],
                                start=(kt == 0), stop=(kt == NKT - 1))
                        tt, po = t0 // 128, t0 % 128
                        if po == 0:
                            if e == "V":
                                nc.vector.tensor_add(vsb[:mw, tt, :],
                                                     pv[:mw, :], bv_t[:mw, :])
                            else:
                                nc.scalar.activation(out=vsb[:mw, tt, :],
                                                     in_=pv[:mw, :],
                                                     func=AF.Copy, scale=1.0)
                        else:
                            # tokens 576..640: partition-offset fixup via DMA
                            vs = ev_pool.tile([128, 512], BF, tag="vs")
                            nc.scalar.activation(out=vs[:mw, :],
                                                 in_=pv[:mw, :],
                                                 func=AF.Copy, scale=1.0)
                            nc.gpsimd.dma_start(out=vsb[po:po + mw, tt, :],
                                                in_=vs[:mw, :])
                    # --- vision side tokens 512..576: moving-operand qkv ---
                    if side:
                        psd = []
                        for p in range(3):
                            psd_p = psSd.tile([128, 512], F32, tag=f"psd{p}")
                            psd.append(psd_p)
                        for ktc in range(8):
                            wq_c = wqm_pool.tile([128, 4, 1024], BF,
                                                 tag="wq_c")
                            nc.sync.dma_start(out=wq_c[:], in_=wqm[ktc])
                            for k8 in range(4):
                                kt = 4 * ktc + k8
                                hs_t, k2 = hsrc(ci, kt)
                                st = hs_t[:, k2, 512:576]
                                nc.tensor.matmul(
                                    psd[0][:64, :], st, wq_c[:, k8, 0:512],
                                    start=(kt == 0), stop=(kt == NKT - 1))
                                nc.tensor.matmul(
                                    psd[1][:64, :], st, wq_c[:, k8, 512:1024],
                                    start=(kt == 0), stop=(kt == NKT - 1))
                                wv_mv = wvt[:, kt, :]
                                nc.tensor.matmul(
                                    psd[2][:64, :], st, wv_mv,
                                    start=(kt == 0), stop=(kt == NKT - 1))
                        # v side: bias along free dim, straight into vsb
                        nc.vector.tensor_add(vsb[:64, 4, :], psd[2][:64, :],
                                             bv_t[:64, :])
                        # q/k side: evac, transpose to [d, tok], rope
                        sqk = ev_pool.tile([128, 8, 128], BF, tag="sqk")
                        nc.scalar.activation(out=sqk[:64, 0:4, :],
                                             in_=psd[0][:64, :],
                                             func=AF.Copy, scale=1.0)
                        nc.scalar.activation(out=sqk[:64, 4:8, :],
                                             in_=psd[1][:64, :],
                                             func=AF.Copy, scale=1.0)
                        for m in range(8):
                            tp = psT.tile([128, 64], BF, tag="tp")
                            nc.tensor.transpose(tp[:], sqk[:64, m, :],
                                                idm_t[:])
                            qs2 = ev_pool.tile([128, 64], BF, tag="qs2")
                            nc.scalar.activation(out=qs2[:], in_=tp[:],
                                                 func=AF.Identity,
                                                 bias=bqk_t[:, m:m + 1],
                                                 scale=1.0)
                            rot2 = psR.tile([128, 512], F32, tag="rot")
                            nc.tensor.matmul(rot2[:, :64], rm_t[:], qs2[:],
                                             start=True, stop=True)
                            tb2 = ev_pool.tile([128, 64], BF, tag="tb2")
                            nc.vector.tensor_mul(tb2[:], rot2[:, :64],
                                                 sin_t[:, 512:NV])
                            qc2 = ev_pool.tile([128, 64], BF, tag="qc2")
                            nc.vector.tensor_mul(qc2[:], qs2[:],
                                                 cos_t[:, 512:NV])
                            nc.vector.tensor_add(qk[:, m, 512:NV], qc2[:],
                                                 tb2[:])

            # ------------- attention + dense phases -------------
            with tc.tile_pool(name="wd", bufs=1) as wd_pool, \
                 tc.tile_pool(name="an", bufs=1) as an_pool:
                # prefetch dense weights during attention
                wd_t = wd_pool.tile([128, 2, 2, HPC, 2048], BF, tag="wd")
                nc.scalar.dma_start(out=wd_t[:, 0], in_=wdw[0])
                nc.scalar.dma_start(out=wd_t[:, 1], in_=wdw[1])
                attn_c = []
                for cc in range(4):
                    attn_t = an_pool.tile([128, HPC, 512], BF, tag=f"at{cc}")
                    attn_c.append(attn_t)
                with tc.tile_pool(name="acns", bufs=1) as acns, \
                     tc.tile_pool(name="pr", bufs=3) as pr_pool, \
                     tc.tile_pool(name="pa", bufs=2) as pa_pool, \
                     tc.tile_pool(name="sm", bufs=2) as sm_pool, \
                     tc.tile_pool(name="oe", bufs=3) as oe_pool, \
                     tc.tile_pool(name="psS", bufs=3, space="PSUM") as psS, \
                     tc.tile_pool(name="psP", bufs=2, space="PSUM") as psP, \
                     tc.tile_pool(name="psU", bufs=1, space="PSUM") as psU, \
                     tc.tile_pool(name="psD", bufs=1, space="PSUM") as psD:
                    mask_t = acns.tile([128, 4, 512], BF)
                    nc.scalar.dma_start(out=mask_t[:], in_=masks[:])
                    ones_t = acns.tile([128, 128], F32R)
                    nc.scalar.dma_start(out=ones_t[:], in_=ones_r[:])
                    for _ in range(3):
                        pb0 = pr_pool.tile([128, 512], BF, tag="pb")
                        nc.vector.memset(pb0[:], 0.0)

                    # dense work: one unit = one hh-matmul of a po group
                    # ((range, nh, n) accumulated over hh); interleaved into
                    # attention's exp-wait bubbles once the needed attn
                    # token-chunk is complete.
                    ranges = []
                    for (t0, t1) in _token_tiles(0, S):
                        if t0 < NV < t1:
                            ranges.append((t0, NV, 0))
                            ranges.append((NV, t1, 1))
                        else:
                            ranges.append((t0, t1, 0 if t0 < NV else 1))
                    units = []
                    for (t0, t1, ei) in ranges:
                        for nh in range(2):
                            for n in range(4):
                                for hh in range(HPC):
                                    units.append((t0 // 512, t0, t1, ei,
                                                  nh, n, hh))
                    dstate = {"gi": 0, "po": None, "ob": None}

                    def dense_step():
                        i = dstate["gi"]
                        if i >= len(units):
                            return False
                        (_, t0, t1, ei, nh, n, hh) = units[i]
                        mw = t1 - t0
                        if units[i][4] == 0 and n == 0 and hh == 0:
                            ob_t = oe_pool.tile([128, HID], BF, tag="ob")
                            dstate["ob"] = ob_t
                        if hh == 0:
                            po_t = psD.tile([128, 512], F32, tag=f"po{n % 2}")
                            dstate["po"] = po_t
                        po, ob = dstate["po"], dstate["ob"]
                        nc.tensor.matmul(
                            po[:mw, :],
                            attn_c[t0 // 512][:, hh,
                                              t0 - 512 * (t0 // 512):
                                              t1 - 512 * (t0 // 512)],
                            wd_t[:, ei, nh, hh, 512 * n:512 * (n + 1)],
                            start=(hh == 0), stop=(hh == HPC - 1))
                        if hh == HPC - 1:
                            dst = ob[:mw, 2048 * nh + 512 * n:
                                     2048 * nh + 512 * (n + 1)]
                            if n % 2 == 0:
                                nc.scalar.activation(out=dst, in_=po[:mw, :],
                                                     func=AF.Copy, scale=1.0)
                            else:
                                nc.vector.tensor_copy(dst, po[:mw, :])
                            if nh == 1 and n == 3:
                                nc.sync.dma_start(out=out_d[t0:t1, :],
                                                  in_=ob[:mw, :])
                        dstate["gi"] = i + 1
                        return True

                    def dense_ready(cur_c):
                        i = dstate["gi"]
                        return i < len(units) and units[i][0] < cur_c

                    for c in range(4):
                        for h in range(HPC):
                            nj = 4 * c + 4
                            ap_ps = psP.tile([128, 512], F32, tag="ap")
                            pacc = pa_pool.tile([128, 512], F32R, tag="pacc")
                            pacv = pa_pool.tile([128, 512], F32R, tag="pacv")
                            for j in range(nj):
                                # diagonal blocks: only cols >= 128r survive
                                r = j - 4 * c
                                x0 = 128 * r if r > 0 else 0
                                scp = psS.tile([128, 512], F32, tag="sc")
                                nc.tensor.matmul(
                                    scp[:, x0:],
                                    qk[:, 4 + h, 128 * j:128 * (j + 1)],
                                    qk[:, h, 512 * c + x0:512 * (c + 1)],
                                    start=True, stop=True)
                                pb = pr_pool.tile([128, 512], BF, tag="pb")
                                nc.scalar.activation(out=pb[:, x0:],
                                                     in_=scp[:, x0:],
                                                     func=AF.Exp, scale=SCALE)
                                if r >= 0:
                                    nc.vector.tensor_mul(
                                        pb[:], pb[:],
                                        mask_t[:, r, :])
                                eng = nc.gpsimd if j % 2 == 0 else nc.vector
                                pac = pacc if j % 2 == 0 else pacv
                                if j < 2:
                                    eng.tensor_copy(pac[:], pb[:])
                                else:
                                    eng.tensor_add(pac[:, x0:], pac[:, x0:],
                                                   pb[:, x0:])
                                if dense_ready(c):
                                    dense_step()
                                if dense_ready(c):
                                    dense_step()
                                if c == 3 and dense_ready(c):
                                    dense_step()
                                nc.tensor.matmul(
                                    ap_ps[:, x0:],
                                    vsb[:, j, 128 * h:128 * (h + 1)],
                                    pb[:, x0:],
                                    start=(j == 0), stop=(j == nj - 1))
                            sp_ps = psU.tile([128, 512], F32, tag="sp")
                            nc.tensor.matmul(sp_ps[:], ones_t[:], pacc[:],
                                             start=True, stop=False)
                            nc.tensor.matmul(sp_ps[:], ones_t[:], pacv[:],
                                             start=False, stop=True)
                            rb = sm_pool.tile([128, 512], F32, tag="rb")
                            nc.vector.reciprocal_approx_fast(out=rb[:],
                                                             in_=sp_ps[:])
                            nc.vector.tensor_mul(
                                attn_c[c][:, h, :], ap_ps[:], rb[:])
                    while dense_step():
                        pass
    nc.compile()
    return nc


def _prep_inputs(inputs):
    hs = np.asarray(inputs["hidden_states"], np.float32)
    cos = np.asarray(inputs["cos"], np.float32)
    sin = np.asarray(inputs["sin"], np.float32)
    vi = np.asarray(inputs["vision_indices"]).ravel()
    li = np.asarray(inputs["language_indices"]).ravel()
    nv = vi.size
    assert nv == NV and np.array_equal(vi, np.arange(nv)) and \
        np.array_equal(li, np.arange(nv, S)), "unsupported index layout"

    # hs tiled per (chunk, kt-quarter): hsq[ci, qi, p, k8, t] =
    # hs[c0 + t, 128*(8*qi + k8) + p]
    hsT = hs.T.astype(BF_NP)
    hsq = np.zeros((4, 4, 128, 8, 576), BF_NP)
    for ci, (c0, c1, _e, side) in enumerate(CHUNKS):
        ww = (c1 - c0) + (64 if side else 0)
        blk = hsT[:, c0:c0 + ww].reshape(4, 8, 128, ww)
        hsq[ci, :, :, :, :ww] = blk.transpose(0, 2, 1, 3)
    cosT = np.ascontiguousarray(cos.T).astype(BF_NP)
    sinT = np.ascontiguousarray(sin.T).astype(BF_NP)
    rmT = np.zeros((D, D), np.float32)
    for d in range(64):
        rmT[d + 64, d] = -1.0
        rmT[d, d + 64] = 1.0
    masks = np.zeros((128, 4, 512), np.float32)
    tri = np.tril(np.ones((128, 128), np.float32)).T  # [t, s]: 1 iff t <= s
    for r in range(4):
        blk = np.ones((128, 512), np.float32)
        blk[:, :128 * r] = 0.0
        blk[:, 128 * r:128 * (r + 1)] = tri
        masks[:, r, :] = blk
    b = np.asarray(inputs["vision_qkv_b"], np.float32)
    wq_all = {"V": np.asarray(inputs["vision_qkv_w"], np.float32),
              "L": np.asarray(inputs["lang_qkv_w"], np.float32)}
    wd_all = {"V": np.asarray(inputs["vision_dense_w"], np.float32),
              "L": np.asarray(inputs["lang_dense_w"], np.float32)}

    def qk_cols(W, r):
        cols = []
        for m in range(8):
            col0 = (0 if m < 4 else HID) + VC * r + 128 * (m % 4)
            cols.append(W[:, col0:col0 + 128])
        return np.stack(cols, 0)                   # [8, HID, 128]

    def qk_tiles(W, r):
        # stationary layout [8, 128, NKT, 128]
        arr = qk_cols(W, r)
        return np.ascontiguousarray(
            arr.reshape(8, NKT, 128, 128).transpose(0, 2, 1, 3)).astype(BF_NP)

    def qm_tiles(W, r):
        # moving layout for the side tokens: [8 ktc, 128, 4 k8, 1024]
        arr = qk_cols(W, r)                        # [8, HID, 128]
        arr = arr.transpose(1, 0, 2).reshape(HID, 1024)   # [HID, 8*128]
        return np.ascontiguousarray(
            arr.reshape(8, 4, 128, 1024).transpose(0, 2, 1, 3)).astype(BF_NP)

    def v_tiles(W, r):
        # [128, NKT, VC]
        c0 = 2 * HID + VC * r
        return np.ascontiguousarray(
            W[:, c0:c0 + VC].reshape(NKT, 128, VC).transpose(1, 0, 2)
        ).astype(BF_NP)

    def d_tiles(Wv, Wl, r):
        # [2(expert), 128, 2(nh), HPC, 2048]
        out = np.empty((2, 128, 2, HPC, 2048), np.float32)
        for ei, W in enumerate((Wv, Wl)):
            rows = W[VC * r:VC * r + VC, :]        # [512, 4096]
            blk = rows.reshape(HPC, 128, 2, 2048)  # [hh, p, nh, c]
            out[ei] = blk.transpose(1, 2, 0, 3)
        return np.ascontiguousarray(out).astype(BF_NP)

    in_maps = []
    for r in range(NCORES):
        bqk_r = np.empty((128, 8), np.float32)
        for m in range(8):
            col0 = (0 if m < 4 else HID) + VC * r + 128 * (m % 4)
            bqk_r[:, m] = b[col0:col0 + 128]
        in_maps.append({
            "hsq": hsq,
            "wqk_v": qk_tiles(wq_all["V"], r),
            "wqk_l": qk_tiles(wq_all["L"], r),
            "wv_v": v_tiles(wq_all["V"], r),
            "wv_l": v_tiles(wq_all["L"], r),
            "wqm": qm_tiles(wq_all["V"], r),
            "wdw": d_tiles(wd_all["V"], wd_all["L"], r),
            "bqk": bqk_r,
            "bv": np.ascontiguousarray(
                b[2 * HID + VC * r:2 * HID + VC * r + VC].reshape(1, VC)),
            "cosw": cosT, "sinw": sinT,
            "rmT": rmT.astype(BF_NP),
            "idm": np.eye(64, dtype=BF_NP),
            "ones": np.ones((128, 128), BF_NP),
            "ones_r": np.ones((128, 128), np.float32),
            "masks": masks.astype(BF_NP),
        })
    return in_maps


def kernel(**inputs):
    if "nc" not in _CACHE:
        _CACHE["nc"] = _build()
    nc = _CACHE["nc"]
    in_maps = _prep_inputs(inputs)
    res = run_bass_kernel_spmd(nc, in_maps, list(range(NCORES)),
                               **_CACHE.get("run_kwargs", {}))
    _CACHE["last_results"] = res
    out = np.zeros((S, HID), np.float64)
    for r in range(NCORES):
        out += res.results[r]["out"].astype(np.float64)
    return out.astype(np.float32)
